# revision 36
# baseline (speedup 1.0000x reference)
# Multi-head causal self-attention (B=2, S=2048, D=768, H=12) on 8 NeuronCores.
#
# Sharding: (batch, head-group) across cores. Core c handles batch c//4 and
# heads 3*(c%4) .. 3*(c%4)+2. Each core computes its heads' Q/K/V projections
# (column-sharded), the causal attention for those heads, and a row-sharded
# partial of the output projection. Host sums the 4 partials per batch + bo.
#
# v3 design notes:
#  - Scores matmuls (K=HD=64) run as row-tiled pairs at PE tile positions
#    (0,0)/(64,0): even key-chunks' K^T lives on SBUF partitions 0-63, odd
#    chunks' on 64-127, Q^T duplicated on both halves.
#  - Attention is a two-slot round-robin over 12 (chunk, head) streams with
#    cross-chunk pairing, so the exp-heavy late chunks overlap earlier ones.
#  - QKV projection of later chunks and the output projection of finished
#    chunks are filler units, popped whenever the PE is predicted to starve
#    (keeps the PE dense: HAM re-throttles on ~0.7us gaps).
#  - k-bias dropped entirely (adds a per-query constant to scores: cancels
#    in softmax); q-bias folded into the Q psum->SBUF copy (tensor_scalar).
#  - Causal masking multiplies only the [128,128] diagonal triangle.
#  - Softmax norm: 1/Z reciprocal on DVE, bf16 cast on Pool, ones-matmul
#    partition broadcast on PE, ACT copy to SBUF, DVE multiply.
#
# All matmul operands are bf16 (fp32 matmuls run the PE array twice per
# instruction); accumulation stays fp32 in PSUM and softmax runs in fp32.
#
# Self-contained: hardcodes shapes; builds the Bass module once per process.

import sys

import ml_dtypes
import numpy as np

sys.path.insert(0, "/opt/trn_rl_repo")

import concourse.bass as bass  # noqa: E402
import concourse.mybir as mybir  # noqa: E402
import concourse.tile as tile  # noqa: E402
from concourse.bass import ts  # noqa: E402
from concourse.bass_utils import run_bass_kernel_spmd  # noqa: E402

F32 = mybir.dt.float32
BF16 = mybir.dt.bfloat16
AF = mybir.ActivationFunctionType
NPBF16 = ml_dtypes.bfloat16

B, S, D, H, HD = 2, 2048, 768, 12, 64
HPC = 3               # heads per core
DQK = 2 * HPC * HD    # 384: per-head-interleaved [Q_h | K_h] projection width
DV = HPC * HD         # 192
P = 128
IC = S // 512         # 4 query chunks of 512
KC = D // P           # 6 contraction chunks
NIO = S // P          # 16 token chunks of 128
NG = S // 256         # 8 groups of 256 keys (even/odd 128-chunk pairs)
N_WARM = 28

# attention stream order: (chunk, head), two streams active at a time
STREAMS = [(0, 2), (0, 0), (0, 1), (1, 2), (1, 0), (1, 1),
           (2, 2), (3, 2), (2, 0), (3, 0), (2, 1), (3, 1)]

# scheduler cost estimates (ns)
PE_MM = lambda n: n / 2.4 + 15
ACT_EXP = lambda free: free * 0.833 + 330
OUT_SPACING = 1600    # min est-PE ns between outproj fillers (DVE copy chain)


def _split_excess_waits(nc, max_waits=1):
    # walrus in this env rejects instructions carrying more than ~1-2
    # sync-waits. Move excess waits onto preceding same-engine nops
    # (sequencer executes the nop's wait, then the instruction's).
    n_split = 0
    for func in nc.m.functions:
        for blk in func.blocks:
            insts = blk.instructions
            out = []
            changed = False
            for inst in insts:
                si = inst.sync_info
                waits = list(si.on_wait) if si and si.on_wait else []
                if len(waits) > max_waits:
                    changed = True
                    for j, w in enumerate(waits[:-max_waits]):
                        out.append(
                            mybir.InstNoOp(
                                name=f"{inst.name}-wsplit{j}",
                                engine=inst.engine,
                                ins=[],
                                outs=[],
                                sync_info=mybir.SyncInfo(
                                    on_wait=[w], on_update=[]
                                ),
                            )
                        )
                        n_split += 1
                    inst.sync_info = mybir.SyncInfo(
                        on_wait=waits[-max_waits:],
                        on_update=list(si.on_update) if si.on_update else [],
                    )
                out.append(inst)
            if changed:
                blk.instructions = out
    return n_split


MM_PHASES = []  # phase tag per emitted matmul, in emission order (debug aid)


def _build_module():
    MM_PHASES.clear()
    nc = bass.Bass()
    xt_d = nc.dram_tensor("xt", [D, S], BF16, kind="ExternalInput")
    wqk_d = nc.dram_tensor("wqk", [D, DQK], BF16, kind="ExternalInput")
    bqs_d = nc.dram_tensor("bqs", [HD, HPC], F32, kind="ExternalInput")
    wv_d = nc.dram_tensor("wv", [D, DV], BF16, kind="ExternalInput")
    wos_d = nc.dram_tensor("wos", [HD, HPC, D], BF16, kind="ExternalInput")
    mask_d = nc.dram_tensor("mask", [P, P], BF16, kind="ExternalInput")
    out_d = nc.dram_tensor("out", [S, D], BF16, kind="ExternalOutput")
    scratch_d = nc.dram_tensor("scratch", [P, 512], F32)

    with tile.TileContext(nc) as tc:
        with (
            tc.tile_pool(name="const", bufs=1) as cp,
            tc.tile_pool(name="qt", bufs=3) as qtp,
            tc.tile_pool(name="exp", bufs=6) as exp_p,
            tc.tile_pool(name="zf", bufs=3) as zfp,
            tc.tile_pool(name="zbc", bufs=3) as zbp,
            tc.tile_pool(name="outp", bufs=3) as op,
            tc.tile_pool(name="proj", bufs=2, space="PSUM") as proj_p,
            tc.tile_pool(name="scps", bufs=2, space="PSUM") as sc_p,
            tc.tile_pool(name="avps", bufs=2, space="PSUM") as av_p,
        ):
            # ---- input DMAs, spread over three DGE queues ----
            xt_sb = cp.tile([P, KC, S], BF16)
            xt_r = xt_d.rearrange("(kc p) t -> p kc t", p=P)
            dma_engs = [nc.sync, nc.scalar]
            wqk_sb = cp.tile([P, KC, DQK], BF16)
            nc.sync.dma_start(wqk_sb, wqk_d.rearrange("(kc p) d -> p kc d", p=P))
            for kc in range(KC):
                dma_engs[kc % 2].dma_start(xt_sb[:, kc, :], xt_r[:, kc, :])
            bqs_sb = cp.tile([HD, HPC], F32)
            nc.scalar.dma_start(bqs_sb, bqs_d[:])
            mask_sb = cp.tile([P, P], BF16)
            nc.scalar.dma_start(mask_sb, mask_d[:])
            wv_sb = cp.tile([P, KC, DV], BF16)
            nc.sync.dma_start(wv_sb, wv_d.rearrange("(kc p) d -> p kc d", p=P))
            wos_sb = cp.tile([HD, HPC, D], BF16)
            nc.scalar.dma_start(wos_sb, wos_d[:])

            ones_sb = cp.tile([1, 512], BF16)
            nc.gpsimd.memset(ones_sb, 1.0)

            # warm up the PE (HAM un-throttle) while input DMAs land:
            # data-independent K=1 matmuls on the memset ones tile.
            warm_ps = proj_p.tile([P, 512], F32, tag="proj")
            for w in range(N_WARM):
                nc.tensor.matmul(
                    warm_ps,
                    lhsT=ones_sb[0:1, 0:P],
                    rhs=ones_sb[0:1, :],
                    start=(w == 0),
                    stop=(w == N_WARM - 1),
                )
            warm_sb = cp.tile([P, 512], F32)
            nc.vector.tensor_copy(warm_sb, warm_ps)
            nc.sync.dma_start(scratch_d[:], warm_sb)
            MM_PHASES.extend(["warm"] * N_WARM)

            # ---- resident SBUF tensors ----
            # klo: per-head K^T; group g keys [256g,256g+128) on partitions
            # 0-63, keys [256g+128, 256(g+1)) on partitions 64-127.
            klo = cp.tile([P, HPC, NG, P], BF16)
            # V plus a ones column (col HD) for the softmax denominator
            v1 = cp.tile([P, NIO, HPC, HD + 1], BF16)
            nc.gpsimd.memset(v1, 1.0)
            ctxT = cp.tile([HD, HPC, S], BF16)    # normalized ctx^T [d, h, i]

            # scheduler state: estimated cumulative engine busy times
            est = {"pe": 0.0, "act": 0.0, "last_out": -1e9}
            act_done = {}  # (ic, h, jb) -> estimated exp completion

            def mm(phase, *a, **kw):
                MM_PHASES.append(phase)
                nc.tensor.matmul(*a, **kw)

            # ---- emission units ----
            qt_tiles = {}

            def qk_unit(ic, h):
                # Q/K projection for 512 tokens of chunk ic, head h.
                if ic not in qt_tiles:
                    qt_tiles[ic] = qtp.tile(
                        [P, HPC, 512], BF16, tag="qt", name=f"qt{ic}"
                    )
                qt = qt_tiles[ic]
                isl = ts(ic, 512)
                ps = proj_p.tile([P, 512], F32, tag="proj")
                for kc in range(KC):
                    mm(
                        f"qk{ic}",
                        ps,
                        lhsT=wqk_sb[:, kc, ts(h, P)],
                        rhs=xt_sb[:, kc, isl],
                        start=(kc == 0),
                        stop=(kc == KC - 1),
                    )
                est["pe"] += 6 * PE_MM(512)
                # Q^T (+bias): DVE reads PSUM into the lower half, Pool
                # duplicates SBUF->SBUF into the upper half.
                nc.vector.tensor_scalar(
                    qt[0:HD, h, :], ps[0:HD, :], bqs_sb[:, h : h + 1], None,
                    mybir.AluOpType.add,
                )
                nc.gpsimd.tensor_copy(qt[HD:P, h, :], qt[0:HD, h, :])
                # K^T into even/odd partition halves per 256-group
                # (strided view: parity dim selects even/odd 128-token chunk)
                kv = ps[HD:P, :].rearrange("p (g two t) -> p g two t", two=2, t=P)
                ksl = klo[:, h, :, :]
                nc.vector.tensor_copy(
                    ksl[0:HD, 2 * ic : 2 * ic + 2, :], kv[:, :, 0, :]
                )
                nc.vector.tensor_copy(
                    ksl[HD:P, 2 * ic : 2 * ic + 2, :], kv[:, :, 1, :]
                )

            def v_unit(ic, io4):
                io = ic * 4 + io4
                ps = proj_p.tile([P, 512], F32, tag="proj")
                psv = ps[:, :DV]
                for kc in range(KC):
                    mm(
                        f"v{ic}",
                        psv,
                        lhsT=xt_sb[:, kc, ts(io, P)],
                        rhs=wv_sb[:, kc, :],
                        start=(kc == 0),
                        stop=(kc == KC - 1),
                    )
                est["pe"] += 6 * PE_MM(192)
                nc.vector.tensor_copy(
                    v1[:, io, :, 0:HD],
                    psv.rearrange("p (h e) -> p h e", e=HD),
                )

            out_tiles = {}

            def outproj_unit(ic, io4, ot):
                io = ic * 4 + io4
                ow = 512 if ot == 0 else 256
                if ot == 0:
                    out_tiles[io] = op.tile(
                        [P, D], BF16, tag="osb", name=f"osb{io}"
                    )
                o_sb = out_tiles[io]
                ps = proj_p.tile([P, 512], F32, tag="proj")
                pso = ps[:, :ow]
                for h in range(HPC):
                    mm(
                        f"out{ic}",
                        pso,
                        lhsT=ctxT[:, h, ts(io, P)],
                        rhs=wos_sb[:, h, ot * 512 : ot * 512 + ow],
                        start=(h == 0),
                        stop=(h == HPC - 1),
                    )
                est["pe"] += 3 * PE_MM(ow)
                est["last_out"] = est["pe"]
                nc.vector.tensor_copy(o_sb[:, ot * 512 : ot * 512 + ow], pso)
                if ot == 1:
                    nc.sync.dma_start(out_d[ts(io, P), :], o_sb)

            # global filler deque: (kind, ic, emit_fn)
            work = []

            def fill_until(target):
                # emit filler units while the PE is predicted to be starved;
                # outproj units are spaced out (their DVE copies pile up
                # behind exp-dependent work and starve the proj PSUM ring)
                while work and est["pe"] + 50 < target:
                    pick = None
                    for i, (kind, ic, fn) in enumerate(work):
                        if kind != "out":
                            pick = i
                            break
                        if est["pe"] - est["last_out"] >= OUT_SPACING:
                            pick = i
                            break
                    if pick is None:
                        break
                    work.pop(pick)[2]()

            def force_units(kind, ic):
                # emit any still-queued units of (kind, ic) right now
                rest = []
                for kind_, ic_, fn in work:
                    if kind_ == kind and ic_ == ic:
                        fn()
                    else:
                        rest.append((kind_, ic_, fn))
                work[:] = rest

            # ---- attention ----
            def trim_of(jc, ic):
                koff = jc - 4 * ic
                return P * koff if koff > 0 else 0

            def emit_scores(ic, h, jb):
                # row-tiled pair: even chunk at PE rows 0-63, odd at 64-127
                qt = qt_tiles[ic]
                g = jb // 2
                t0 = trim_of(jb, ic)
                t1 = trim_of(jb + 1, ic)
                sc = sc_p.tile([P, 2, 512], F32, tag="sc", name=f"sc{ic}{h}{jb}")
                mm(
                    f"sc{ic}",
                    sc[:, 0, t0:],
                    lhsT=klo[0:HD, h, g, :],
                    rhs=qt[0:HD, h, t0:],
                    start=True,
                    stop=True,
                )
                mm(
                    f"sc{ic}",
                    sc[:, 1, t1:],
                    lhsT=klo[HD:P, h, g, :],
                    rhs=qt[HD:P, h, t1:],
                    start=True,
                    stop=True,
                )
                est["pe"] += PE_MM(512 - t0) + 60
                ex = exp_p.tile([P, 2, 512], BF16, tag="ex", name=f"ex{ic}{h}{jb}")
                koff = jb - 4 * ic
                if koff >= 0:
                    # diagonal pair: exp the trimmed range, then mask only
                    # the [128,128] triangle at the front of each sub-chunk
                    a_free = 0
                    for k in range(2):
                        t = trim_of(jb + k, ic)
                        nc.scalar.activation(ex[:, k, t:], sc[:, k, t:], AF.Exp)
                        nc.vector.tensor_mul(
                            ex[:, k, t : t + P], ex[:, k, t : t + P], mask_sb
                        )
                        a_free += 512 - t
                    est["act"] = max(est["act"], est["pe"]) + ACT_EXP(a_free) + 250
                else:
                    nc.scalar.activation(ex, sc, AF.Exp)
                    est["act"] = max(est["act"], est["pe"]) + ACT_EXP(1024)
                act_done[(ic, h, jb)] = est["act"]
                return ex

            def emit_av(avp, ic, h, ex, jb, n_j):
                for k in range(2):
                    jc = jb + k
                    t = trim_of(jc, ic)
                    mm(
                        f"av{ic}",
                        avp[:, t:],
                        lhsT=v1[:, jc, h, :],
                        rhs=ex[:, k, t:],
                        start=(jc == 0),
                        stop=(jc == n_j - 1),
                    )
                    est["pe"] += PE_MM(512 - t)

            def emit_norm(avp, ic, h):
                # ctxT = avp[0:HD] * (1/Z): DVE reciprocal, Pool cast to
                # bf16, PE ones-matmul broadcast, ACT copy out, DVE multiply
                fill_until(est["pe"] + 4200)
                zf = zfp.tile([1, 512], F32, tag="zf", name=f"zf{ic}{h}")
                nc.vector.reciprocal(zf, avp[HD : HD + 1, :])
                z16 = zbp.tile([1, 512], BF16, tag="z16", name=f"z16_{ic}{h}")
                nc.gpsimd.tensor_copy(z16, zf)
                rb = proj_p.tile([P, 512], F32, tag="proj", name=f"rb{ic}{h}")
                mm(
                    f"rb{ic}",
                    rb[0:HD, :],
                    lhsT=ones_sb[0:1, 0:HD],
                    rhs=z16,
                    start=True,
                    stop=True,
                )
                est["pe"] += PE_MM(512)
                rb_sb = zfp.tile([HD, 512], F32, tag="rbsb", name=f"rs{ic}{h}")
                nc.scalar.copy(rb_sb, rb[0:HD, :])
                nc.vector.tensor_tensor(
                    ctxT[:, h, ts(ic, 512)],
                    avp[0:HD, :],
                    rb_sb,
                    mybir.AluOpType.mult,
                )

            # ---- stream scheduler ----
            for h in range(HPC):
                qk_unit(0, h)
            for io4 in range(4):
                v_unit(0, io4)

            class Stream:
                def __init__(self, ic, h):
                    self.ic, self.h = ic, h
                    self.n_j = 4 * ic + 4
                    self.jbs = list(range(0, self.n_j, 2))
                    self.prev = None
                    self.av = av_p.tile(
                        [HD + 1, 512], F32, tag="av", name=f"av{ic}_{h}"
                    )

            started_ic = set()
            finished = {ic: 0 for ic in range(IC)}
            si = 0
            slots = [None, None]

            def start_next():
                nonlocal si
                if si >= len(STREAMS):
                    return None
                ic, h = STREAMS[si]
                si += 1
                if ic not in started_ic:
                    started_ic.add(ic)
                    if ic + 1 < IC:
                        for hh in range(HPC):
                            work.append(
                                ("qk", ic + 1,
                                 lambda ic=ic, hh=hh: qk_unit(ic + 1, hh)))
                        for io4 in range(4):
                            work.append(
                                ("v", ic + 1,
                                 lambda ic=ic, io4=io4: v_unit(ic + 1, io4)))
                if ic > 0:
                    # prerequisites: this chunk's own projections
                    force_units("qk", ic)
                    force_units("v", ic)
                return Stream(ic, h)

            while True:
                for sl in (0, 1):
                    s = slots[sl]
                    if s is None:
                        s = slots[sl] = start_next()
                        if s is None:
                            continue
                    if s.jbs:
                        jb = s.jbs.pop(0)
                        ex = emit_scores(s.ic, s.h, jb)
                        if s.prev is not None:
                            pex, pjb = s.prev
                            fill_until(act_done[(s.ic, s.h, pjb)])
                            emit_av(s.av, s.ic, s.h, pex, pjb, s.n_j)
                        s.prev = (ex, jb)
                    else:
                        pex, pjb = s.prev
                        fill_until(act_done[(s.ic, s.h, pjb)])
                        emit_av(s.av, s.ic, s.h, pex, pjb, s.n_j)
                        emit_norm(s.av, s.ic, s.h)
                        finished[s.ic] += 1
                        if finished[s.ic] == HPC and s.ic < 3:
                            for io4 in range(4):
                                for ot in range(2):
                                    work.append(
                                        ("out", s.ic,
                                         lambda ic=s.ic, io4=io4, ot=ot:
                                         outproj_unit(ic, io4, ot)))
                        slots[sl] = None
                if si >= len(STREAMS) and slots[0] is None and slots[1] is None:
                    break

            # drain leftover fillers, then the final chunk's output projection
            while work:
                work.pop(0)[2]()
            for io4 in range(4):
                for ot in range(2):
                    outproj_unit(3, io4, ot)

    _split_excess_waits(nc)
    return nc


_NC = None


def _get_nc():
    global _NC
    if _NC is None:
        _NC = _build_module()
    return _NC


def _make_mask():
    p = np.arange(P)[:, None]
    f = np.arange(P)[None, :]
    return (p <= f).astype(np.float32).astype(NPBF16)


def _build_in_maps(x, wq, bq, wk, bk, wv, bv, wo):
    scale = 1.0 / np.sqrt(HD)
    mask = _make_mask()
    in_maps = []
    for core in range(8):
        b = core // 4
        h0 = (core % 4) * HPC
        heads = list(range(h0, h0 + HPC))

        wqk = np.empty((D, DQK), np.float32)
        bqs = np.empty((HD, HPC), np.float32)
        for hl, hg in enumerate(heads):
            cs = slice(hg * HD, (hg + 1) * HD)
            wqk[:, hl * P : hl * P + HD] = wq[:, cs] * scale
            wqk[:, hl * P + HD : (hl + 1) * P] = wk[:, cs]
            bqs[:, hl] = bq[cs] * scale

        vcols = slice(h0 * HD, (h0 + HPC) * HD)
        wos = (
            wo[vcols, :].reshape(HPC, HD, D).transpose(1, 0, 2)
        )  # [HD, HPC, D]

        in_maps.append(
            {
                "xt": np.ascontiguousarray(x[b].T).astype(NPBF16),
                "wqk": wqk.astype(NPBF16),
                "bqs": bqs,
                "wv": np.ascontiguousarray(wv[:, vcols]).astype(NPBF16),
                "wos": np.ascontiguousarray(wos).astype(NPBF16),
                "mask": mask,
            }
        )
    return in_maps


def kernel(x, wq, bq, wk, bk, wv, bv, wo, bo):
    x = np.asarray(x, np.float32)
    wq = np.asarray(wq, np.float32)
    bq = np.asarray(bq, np.float32)
    wk = np.asarray(wk, np.float32)
    bk = np.asarray(bk, np.float32)
    wv = np.asarray(wv, np.float32)
    bv = np.asarray(bv, np.float32)
    wo = np.asarray(wo, np.float32)
    bo = np.asarray(bo, np.float32)

    in_maps = _build_in_maps(x, wq, bq, wk, bk, wv, bv, wo)
    res = run_bass_kernel_spmd(_get_nc(), in_maps, core_ids=list(range(8)))
    out = np.zeros((B, S, D), np.float32)
    for core in range(8):
        out[core // 4] += np.asarray(res.results[core]["out"], np.float32)
    out += bo + bv @ wo
    return out


# revision 38
# speedup vs baseline: 1.0273x; 1.0273x over previous
# Multi-head causal self-attention (B=2, S=2048, D=768, H=12) on 8 NeuronCores.
#
# Sharding: (batch, head-group) across cores. Core c handles batch c//4 and
# heads 3*(c%4) .. 3*(c%4)+2. Each core computes its heads' Q/K/V projections
# (column-sharded), the causal attention for those heads, and a row-sharded
# partial of the output projection. Host sums the 4 partials per batch + bo.
#
# v3 design notes:
#  - Scores matmuls (K=HD=64) run as row-tiled pairs at PE tile positions
#    (0,0)/(64,0): even key-chunks' K^T lives on SBUF partitions 0-63, odd
#    chunks' on 64-127, Q^T duplicated on both halves.
#  - Attention is a two-slot round-robin over 12 (chunk, head) streams with
#    cross-chunk pairing, so the exp-heavy late chunks overlap earlier ones.
#  - QKV projection of later chunks and the output projection of finished
#    chunks are filler units, popped whenever the PE is predicted to starve
#    (keeps the PE dense: HAM re-throttles on ~0.7us gaps).
#  - k-bias dropped entirely (adds a per-query constant to scores: cancels
#    in softmax); q-bias folded into the Q psum->SBUF copy (tensor_scalar).
#  - Causal masking multiplies only the [128,128] diagonal triangle.
#  - Softmax norm: 1/Z reciprocal on DVE, bf16 cast on Pool, ones-matmul
#    partition broadcast on PE, ACT copy to SBUF, DVE multiply.
#
# All matmul operands are bf16 (fp32 matmuls run the PE array twice per
# instruction); accumulation stays fp32 in PSUM and softmax runs in fp32.
#
# Self-contained: hardcodes shapes; builds the Bass module once per process.

import sys

import ml_dtypes
import numpy as np

sys.path.insert(0, "/opt/trn_rl_repo")

import concourse.bass as bass  # noqa: E402
import concourse.mybir as mybir  # noqa: E402
import concourse.tile as tile  # noqa: E402
from concourse.bass import ts  # noqa: E402
from concourse.bass_utils import run_bass_kernel_spmd  # noqa: E402

F32 = mybir.dt.float32
BF16 = mybir.dt.bfloat16
AF = mybir.ActivationFunctionType
NPBF16 = ml_dtypes.bfloat16

B, S, D, H, HD = 2, 2048, 768, 12, 64
HPC = 3               # heads per core
DQK = 2 * HPC * HD    # 384: per-head-interleaved [Q_h | K_h] projection width
DV = HPC * HD         # 192
P = 128
IC = S // 512         # 4 query chunks of 512
KC = D // P           # 6 contraction chunks
NIO = S // P          # 16 token chunks of 128
NG = S // 256         # 8 groups of 256 keys (even/odd 128-chunk pairs)
N_WARM = 28

# attention stream order: (chunk, head), two streams active at a time
STREAMS = [(0, 2), (0, 0), (0, 1), (1, 2), (1, 0), (1, 1),
           (2, 2), (3, 2), (2, 0), (3, 0), (2, 1), (3, 1)]

# scheduler cost estimates (ns)
PE_MM = lambda n: n / 2.4 + 15
ACT_EXP = lambda free: free * 0.833 + 330
OUT_SPACING = 1600    # min est-PE ns between outproj fillers (DVE copy chain)


def _split_excess_waits(nc, max_waits=1):
    # walrus in this env rejects instructions carrying more than ~1-2
    # sync-waits. Move excess waits onto preceding same-engine nops
    # (sequencer executes the nop's wait, then the instruction's).
    n_split = 0
    for func in nc.m.functions:
        for blk in func.blocks:
            insts = blk.instructions
            out = []
            changed = False
            for inst in insts:
                si = inst.sync_info
                waits = list(si.on_wait) if si and si.on_wait else []
                if len(waits) > max_waits:
                    changed = True
                    for j, w in enumerate(waits[:-max_waits]):
                        out.append(
                            mybir.InstNoOp(
                                name=f"{inst.name}-wsplit{j}",
                                engine=inst.engine,
                                ins=[],
                                outs=[],
                                sync_info=mybir.SyncInfo(
                                    on_wait=[w], on_update=[]
                                ),
                            )
                        )
                        n_split += 1
                    inst.sync_info = mybir.SyncInfo(
                        on_wait=waits[-max_waits:],
                        on_update=list(si.on_update) if si.on_update else [],
                    )
                out.append(inst)
            if changed:
                blk.instructions = out
    return n_split


MM_PHASES = []  # phase tag per emitted matmul, in emission order (debug aid)


def _build_module():
    MM_PHASES.clear()
    nc = bass.Bass()
    xt_d = nc.dram_tensor("xt", [D, S], BF16, kind="ExternalInput")
    wqk_d = nc.dram_tensor("wqk", [D, DQK], BF16, kind="ExternalInput")
    bqs_d = nc.dram_tensor("bqs", [HD, HPC], F32, kind="ExternalInput")
    wv_d = nc.dram_tensor("wv", [D, DV], BF16, kind="ExternalInput")
    wos_d = nc.dram_tensor("wos", [HD, HPC, D], BF16, kind="ExternalInput")
    mask_d = nc.dram_tensor("mask", [P, P], BF16, kind="ExternalInput")
    out_d = nc.dram_tensor("out", [S, D], BF16, kind="ExternalOutput")
    scratch_d = nc.dram_tensor("scratch", [P, 512], F32)

    with tile.TileContext(nc) as tc:
        with (
            tc.tile_pool(name="const", bufs=1) as cp,
            tc.tile_pool(name="qt", bufs=3) as qtp,
            tc.tile_pool(name="exp", bufs=6) as exp_p,
            tc.tile_pool(name="zf", bufs=3) as zfp,
            tc.tile_pool(name="zbc", bufs=3) as zbp,
            tc.tile_pool(name="outp", bufs=3) as op,
            tc.tile_pool(name="proj", bufs=2, space="PSUM") as proj_p,
            tc.tile_pool(name="scps", bufs=2, space="PSUM") as sc_p,
            tc.tile_pool(name="avps", bufs=2, space="PSUM") as av_p,
        ):
            # ---- input DMAs, spread over three DGE queues ----
            xt_sb = cp.tile([P, KC, S], BF16)
            xt_r = xt_d.rearrange("(kc p) t -> p kc t", p=P)
            dma_engs = [nc.sync, nc.scalar]
            wqk_sb = cp.tile([P, KC, DQK], BF16)
            nc.sync.dma_start(wqk_sb, wqk_d.rearrange("(kc p) d -> p kc d", p=P))
            for kc in range(KC):
                dma_engs[kc % 2].dma_start(xt_sb[:, kc, :], xt_r[:, kc, :])
            bqs_sb = cp.tile([HD, HPC], F32)
            nc.scalar.dma_start(bqs_sb, bqs_d[:])
            mask_sb = cp.tile([P, P], BF16)
            nc.scalar.dma_start(mask_sb, mask_d[:])
            wv_sb = cp.tile([P, KC, DV], BF16)
            nc.sync.dma_start(wv_sb, wv_d.rearrange("(kc p) d -> p kc d", p=P))
            wos_sb = cp.tile([HD, HPC, D], BF16)
            nc.scalar.dma_start(wos_sb, wos_d[:])

            ones_sb = cp.tile([1, 512], BF16)
            nc.gpsimd.memset(ones_sb, 1.0)
            ones2 = cp.tile([P, 512], BF16)
            nc.gpsimd.memset(ones2, 1.0)
            zeros_sb = cp.tile([P, HD], BF16)
            nc.gpsimd.memset(zeros_sb, 0.0)

            # warm up the PE (HAM un-throttle) while input DMAs land.
            # HAM's activity metric is utilization-weighted: K=1 matmuls
            # do NOT register (v3 trace: 12us of K=1 warm never
            # un-throttled). Use full K=128 x M=128 matmuls.
            warm_ps = proj_p.tile([P, 512], F32, tag="proj")
            for w in range(N_WARM):
                nc.tensor.matmul(
                    warm_ps,
                    lhsT=ones2[:, 0:P],
                    rhs=ones2[:, :],
                    start=(w == 0),
                    stop=(w == N_WARM - 1),
                )
            warm_sb = cp.tile([P, 512], F32)
            nc.vector.tensor_copy(warm_sb, warm_ps)
            nc.sync.dma_start(scratch_d[:], warm_sb)
            MM_PHASES.extend(["warm"] * N_WARM)

            # ---- resident SBUF tensors ----
            # klo: per-head K^T; group g keys [256g,256g+128) on partitions
            # 0-63, keys [256g+128, 256(g+1)) on partitions 64-127.
            klo = cp.tile([P, HPC, NG, P], BF16)
            # V plus a ones column (col HD) for the softmax denominator
            v1 = cp.tile([P, NIO, HPC, HD + 1], BF16)
            nc.gpsimd.memset(v1, 1.0)
            ctxT = cp.tile([HD, HPC, S], BF16)    # normalized ctx^T [d, h, i]

            # scheduler state: estimated cumulative engine busy times
            est = {"pe": 0.0, "act": 0.0, "last_out": -1e9}
            act_done = {}  # (ic, h, jb) -> estimated exp completion

            def mm(phase, *a, **kw):
                MM_PHASES.append(phase)
                nc.tensor.matmul(*a, **kw)

            # ---- emission units ----
            qt_tiles = {}

            def qk_unit(ic, h):
                # Q/K projection for 512 tokens of chunk ic, head h.
                if ic not in qt_tiles:
                    qt_tiles[ic] = qtp.tile(
                        [P, HPC, 512], BF16, tag="qt", name=f"qt{ic}"
                    )
                qt = qt_tiles[ic]
                isl = ts(ic, 512)
                ps = proj_p.tile([P, 512], F32, tag="proj")
                for kc in range(KC):
                    mm(
                        f"qk{ic}",
                        ps,
                        lhsT=wqk_sb[:, kc, ts(h, P)],
                        rhs=xt_sb[:, kc, isl],
                        start=(kc == 0),
                        stop=(kc == KC - 1),
                    )
                est["pe"] += 6 * PE_MM(512)
                # Q^T (+bias): DVE reads PSUM into the lower half, Pool
                # duplicates SBUF->SBUF into the upper half.
                nc.vector.tensor_scalar(
                    qt[0:HD, h, :], ps[0:HD, :], bqs_sb[:, h : h + 1], None,
                    mybir.AluOpType.add,
                )
                nc.gpsimd.tensor_copy(qt[HD:P, h, :], qt[0:HD, h, :])
                # K^T into even/odd partition halves per 256-group
                # (strided view: parity dim selects even/odd 128-token chunk)
                kv = ps[HD:P, :].rearrange("p (g two t) -> p g two t", two=2, t=P)
                ksl = klo[:, h, :, :]
                nc.vector.tensor_copy(
                    ksl[0:HD, 2 * ic : 2 * ic + 2, :], kv[:, :, 0, :]
                )
                nc.vector.tensor_copy(
                    ksl[HD:P, 2 * ic : 2 * ic + 2, :], kv[:, :, 1, :]
                )

            def v_unit(ic, io4):
                io = ic * 4 + io4
                ps = proj_p.tile([P, 512], F32, tag="proj")
                psv = ps[:, :DV]
                for kc in range(KC):
                    mm(
                        f"v{ic}",
                        psv,
                        lhsT=xt_sb[:, kc, ts(io, P)],
                        rhs=wv_sb[:, kc, :],
                        start=(kc == 0),
                        stop=(kc == KC - 1),
                    )
                est["pe"] += 6 * PE_MM(192)
                nc.vector.tensor_copy(
                    v1[:, io, :, 0:HD],
                    psv.rearrange("p (h e) -> p h e", e=HD),
                )

            out_tiles = {}

            def outproj_unit(ic, io4, ot):
                io = ic * 4 + io4
                ow = 512 if ot == 0 else 256
                if ot == 0:
                    out_tiles[io] = op.tile(
                        [P, D], BF16, tag="osb", name=f"osb{io}"
                    )
                o_sb = out_tiles[io]
                ps = proj_p.tile([P, 512], F32, tag="proj")
                pso = ps[:, :ow]
                for h in range(HPC):
                    mm(
                        f"out{ic}",
                        pso,
                        lhsT=ctxT[:, h, ts(io, P)],
                        rhs=wos_sb[:, h, ot * 512 : ot * 512 + ow],
                        start=(h == 0),
                        stop=(h == HPC - 1),
                    )
                est["pe"] += 3 * PE_MM(ow)
                est["last_out"] = est["pe"]
                nc.vector.tensor_copy(o_sb[:, ot * 512 : ot * 512 + ow], pso)
                if ot == 1:
                    nc.sync.dma_start(out_d[ts(io, P), :], o_sb)

            # global filler deque: (kind, ic, emit_fn)
            work = []
            pad_target = [None]  # an open AV psum accumulation to zero-add

            def emit_pad(rows):
                # zero-adding matmuls (lhsT = zeros) into a live AV psum
                # accumulation: numerically a no-op, but keeps the PE's
                # HAM activity up through cross-engine stalls.
                avp = pad_target[0]
                if avp is None:
                    return
                n = 0
                while n < rows:
                    w = min(512, rows - n)
                    mm(
                        "pad",
                        avp[0:HD, 0:w],
                        lhsT=zeros_sb[:, 0:HD],
                        rhs=ones2[:, 0:w],
                        start=False,
                        stop=False,
                        skip_group_check=True,
                    )
                    est["pe"] += PE_MM(w)
                    n += w

            def fill_until(target, pad=True):
                # emit filler units while the PE is predicted to be starved;
                # outproj units are spaced out (their DVE copies pile up
                # behind exp-dependent work and starve the proj PSUM ring)
                while work and est["pe"] + 50 < target:
                    pick = None
                    for i, (kind, ic, fn) in enumerate(work):
                        if kind != "out":
                            pick = i
                            break
                        if est["pe"] - est["last_out"] >= OUT_SPACING:
                            pick = i
                            break
                    if pick is None:
                        break
                    work.pop(pick)[2]()
                if pad and est["pe"] + 50 < target:
                    emit_pad(int((target - est["pe"]) * 2.4))

            def force_units(kind, ic):
                # emit any still-queued units of (kind, ic) right now
                rest = []
                for kind_, ic_, fn in work:
                    if kind_ == kind and ic_ == ic:
                        fn()
                    else:
                        rest.append((kind_, ic_, fn))
                work[:] = rest

            # ---- attention ----
            def trim_of(jc, ic):
                koff = jc - 4 * ic
                return P * koff if koff > 0 else 0

            def emit_scores(ic, h, jb):
                # row-tiled pair: even chunk at PE rows 0-63, odd at 64-127
                qt = qt_tiles[ic]
                g = jb // 2
                t0 = trim_of(jb, ic)
                t1 = trim_of(jb + 1, ic)
                sc = sc_p.tile([P, 2, 512], F32, tag="sc", name=f"sc{ic}{h}{jb}")
                mm(
                    f"sc{ic}",
                    sc[:, 0, t0:],
                    lhsT=klo[0:HD, h, g, :],
                    rhs=qt[0:HD, h, t0:],
                    start=True,
                    stop=True,
                )
                mm(
                    f"sc{ic}",
                    sc[:, 1, t1:],
                    lhsT=klo[HD:P, h, g, :],
                    rhs=qt[HD:P, h, t1:],
                    start=True,
                    stop=True,
                )
                est["pe"] += PE_MM(512 - t0) + 60
                ex = exp_p.tile([P, 2, 512], BF16, tag="ex", name=f"ex{ic}{h}{jb}")
                koff = jb - 4 * ic
                if koff >= 0:
                    # diagonal pair: exp the trimmed range, then mask only
                    # the [128,128] triangle at the front of each sub-chunk
                    a_free = 0
                    for k in range(2):
                        t = trim_of(jb + k, ic)
                        nc.scalar.activation(ex[:, k, t:], sc[:, k, t:], AF.Exp)
                        nc.vector.tensor_mul(
                            ex[:, k, t : t + P], ex[:, k, t : t + P], mask_sb
                        )
                        a_free += 512 - t
                    est["act"] = max(est["act"], est["pe"]) + ACT_EXP(a_free) + 250
                else:
                    nc.scalar.activation(ex, sc, AF.Exp)
                    est["act"] = max(est["act"], est["pe"]) + ACT_EXP(1024)
                act_done[(ic, h, jb)] = est["act"]
                return ex

            def emit_av(avp, ic, h, ex, jb, n_j):
                for k in range(2):
                    jc = jb + k
                    t = trim_of(jc, ic)
                    mm(
                        f"av{ic}",
                        avp[:, t:],
                        lhsT=v1[:, jc, h, :],
                        rhs=ex[:, k, t:],
                        start=(jc == 0),
                        stop=(jc == n_j - 1),
                    )
                    est["pe"] += PE_MM(512 - t)

            def emit_norm(avp, ic, h):
                # ctxT = avp[0:HD] * (1/Z): DVE reciprocal, Pool cast to
                # bf16, PE ones-matmul broadcast, ACT copy out, DVE multiply
                fill_until(est["pe"] + 4200)
                zf = zfp.tile([1, 512], F32, tag="zf", name=f"zf{ic}{h}")
                nc.vector.reciprocal(zf, avp[HD : HD + 1, :])
                z16 = zbp.tile([1, 512], BF16, tag="z16", name=f"z16_{ic}{h}")
                nc.vector.tensor_copy(z16, zf)
                rb = proj_p.tile([P, 512], F32, tag="proj", name=f"rb{ic}{h}")
                mm(
                    f"rb{ic}",
                    rb[0:HD, :],
                    lhsT=ones_sb[0:1, 0:HD],
                    rhs=z16,
                    start=True,
                    stop=True,
                )
                est["pe"] += PE_MM(512)
                rb_sb = zfp.tile([HD, 512], F32, tag="rbsb", name=f"rs{ic}{h}")
                nc.scalar.copy(rb_sb, rb[0:HD, :])
                nc.vector.tensor_tensor(
                    ctxT[:, h, ts(ic, 512)],
                    avp[0:HD, :],
                    rb_sb,
                    mybir.AluOpType.mult,
                )

            # ---- stream scheduler ----
            for h in range(HPC):
                qk_unit(0, h)
            for io4 in range(4):
                v_unit(0, io4)

            class Stream:
                def __init__(self, ic, h):
                    self.ic, self.h = ic, h
                    self.n_j = 4 * ic + 4
                    self.jbs = list(range(0, self.n_j, 2))
                    self.prev = None
                    self.av = av_p.tile(
                        [HD + 1, 512], F32, tag="av", name=f"av{ic}_{h}"
                    )

            started_ic = set()
            finished = {ic: 0 for ic in range(IC)}
            si = 0
            slots = [None, None]

            def start_next():
                nonlocal si
                if si >= len(STREAMS):
                    return None
                ic, h = STREAMS[si]
                si += 1
                if ic not in started_ic:
                    started_ic.add(ic)
                    if ic + 1 < IC:
                        for hh in range(HPC):
                            work.append(
                                ("qk", ic + 1,
                                 lambda ic=ic, hh=hh: qk_unit(ic + 1, hh)))
                        for io4 in range(4):
                            work.append(
                                ("v", ic + 1,
                                 lambda ic=ic, io4=io4: v_unit(ic + 1, io4)))
                if ic > 0:
                    # prerequisites: this chunk's own projections
                    force_units("qk", ic)
                    force_units("v", ic)
                return Stream(ic, h)

            while True:
                for sl in (0, 1):
                    s = slots[sl]
                    if s is None:
                        s = slots[sl] = start_next()
                        if s is None:
                            continue
                    if s.jbs:
                        pad_target[0] = s.av
                        jb = s.jbs.pop(0)
                        ex = emit_scores(s.ic, s.h, jb)
                        if s.prev is not None:
                            pex, pjb = s.prev
                            fill_until(act_done[(s.ic, s.h, pjb)])
                            emit_av(s.av, s.ic, s.h, pex, pjb, s.n_j)
                        s.prev = (ex, jb)
                    else:
                        other = slots[1 - sl]
                        pad_target[0] = other.av if other is not None else None
                        pex, pjb = s.prev
                        fill_until(act_done[(s.ic, s.h, pjb)])
                        emit_av(s.av, s.ic, s.h, pex, pjb, s.n_j)
                        emit_norm(s.av, s.ic, s.h)
                        finished[s.ic] += 1
                        if finished[s.ic] == HPC and s.ic < 3:
                            for io4 in range(4):
                                for ot in range(2):
                                    work.append(
                                        ("out", s.ic,
                                         lambda ic=s.ic, io4=io4, ot=ot:
                                         outproj_unit(ic, io4, ot)))
                        slots[sl] = None
                if si >= len(STREAMS) and slots[0] is None and slots[1] is None:
                    break

            # drain leftover fillers, then the final chunk's output projection
            while work:
                work.pop(0)[2]()
            for io4 in range(4):
                for ot in range(2):
                    outproj_unit(3, io4, ot)

    _split_excess_waits(nc)
    return nc


_NC = None


def _get_nc():
    global _NC
    if _NC is None:
        _NC = _build_module()
    return _NC


def _make_mask():
    p = np.arange(P)[:, None]
    f = np.arange(P)[None, :]
    return (p <= f).astype(np.float32).astype(NPBF16)


def _build_in_maps(x, wq, bq, wk, bk, wv, bv, wo):
    scale = 1.0 / np.sqrt(HD)
    mask = _make_mask()
    in_maps = []
    for core in range(8):
        b = core // 4
        h0 = (core % 4) * HPC
        heads = list(range(h0, h0 + HPC))

        wqk = np.empty((D, DQK), np.float32)
        bqs = np.empty((HD, HPC), np.float32)
        for hl, hg in enumerate(heads):
            cs = slice(hg * HD, (hg + 1) * HD)
            wqk[:, hl * P : hl * P + HD] = wq[:, cs] * scale
            wqk[:, hl * P + HD : (hl + 1) * P] = wk[:, cs]
            bqs[:, hl] = bq[cs] * scale

        vcols = slice(h0 * HD, (h0 + HPC) * HD)
        wos = (
            wo[vcols, :].reshape(HPC, HD, D).transpose(1, 0, 2)
        )  # [HD, HPC, D]

        in_maps.append(
            {
                "xt": np.ascontiguousarray(x[b].T).astype(NPBF16),
                "wqk": wqk.astype(NPBF16),
                "bqs": bqs,
                "wv": np.ascontiguousarray(wv[:, vcols]).astype(NPBF16),
                "wos": np.ascontiguousarray(wos).astype(NPBF16),
                "mask": mask,
            }
        )
    return in_maps


def kernel(x, wq, bq, wk, bk, wv, bv, wo, bo):
    x = np.asarray(x, np.float32)
    wq = np.asarray(wq, np.float32)
    bq = np.asarray(bq, np.float32)
    wk = np.asarray(wk, np.float32)
    bk = np.asarray(bk, np.float32)
    wv = np.asarray(wv, np.float32)
    bv = np.asarray(bv, np.float32)
    wo = np.asarray(wo, np.float32)
    bo = np.asarray(bo, np.float32)

    in_maps = _build_in_maps(x, wq, bq, wk, bk, wv, bv, wo)
    res = run_bass_kernel_spmd(_get_nc(), in_maps, core_ids=list(range(8)))
    out = np.zeros((B, S, D), np.float32)
    for core in range(8):
        out[core // 4] += np.asarray(res.results[core]["out"], np.float32)
    out += bo + bv @ wo
    return out
